# revision 6
# baseline (speedup 1.0000x reference)
"""Bahdanau additive-attention kernel for Trainium2 (Bass/Tile), 8-core SPMD.

Problem shapes (hardcoded): B=8, S_ENC=256, S_DEC=128, D_ENC=D_DEC=512, UNITS=512.
Sharding: data-parallel over batch B -> one batch element per NeuronCore;
weights replicated.

Math per batch element:
    d_enc = enc @ W_enc + b_enc                    # [256, 512]
    d_dec = dec @ W_dec + b_dec                    # [128, 512]
    scores[q,e] = sum_u tanh(d_dec[q,u] + d_enc[e,u]) * w_score[u]
    weights = softmax(scores, axis=e)              # bias_score cancels in softmax
    out[q,:] = weights[q,:] @ enc                  # [128, 512]

The [128,256,512] tanh intermediate never touches HBM: it is produced in
bf16 SBUF tiles (DVE tensor_scalar broadcast-add at 4x + one large ACT Tanh
per q-block) and consumed immediately by PE as the matmul stationary operand
(bf16 -> fast weight load) against w_score, accumulating scores^T in PSUM.

n_iters > 1 replays the whole pipeline inside one NEFF; used only for
wall-clock-delta timing in test.py.
"""

import numpy as np

import concourse.bass as bass
import concourse.tile as tile
from concourse import bacc, mybir
from concourse.bass_utils import run_bass_kernel_spmd
from concourse.masks import make_identity

F32 = mybir.dt.float32
BF16 = mybir.dt.bfloat16
AF = mybir.ActivationFunctionType

S_ENC, S_DEC, D, U = 256, 128, 512, 512
UC = U // 128      # 4 u-chunks
EC = S_ENC // 128  # 2 e-chunks
DC = D // 128      # 4 d-chunks
QB = 8             # q rows per main-loop block
NBLK = S_DEC // QB

N_CORES = 8


def build_program(n_iters: int = 1):
    """Build the single-core program; SPMD-replicated across 8 cores."""
    nc = bacc.Bacc("TRN2", target_bir_lowering=False, debug=False,
                   num_devices=N_CORES)

    enc_d = nc.dram_tensor("enc", [S_ENC, D], F32, kind="ExternalInput")
    dec_d = nc.dram_tensor("dec", [S_DEC, D], F32, kind="ExternalInput")
    wenc_d = nc.dram_tensor("w_enc", [D, U], F32, kind="ExternalInput")
    wdec_d = nc.dram_tensor("w_dec", [D, U], F32, kind="ExternalInput")
    wsc_d = nc.dram_tensor("w_score", [U, 1], F32, kind="ExternalInput")
    benc_d = nc.dram_tensor("b_enc", [U, 1], F32, kind="ExternalInput")
    bdec_d = nc.dram_tensor("b_dec", [U, 1], F32, kind="ExternalInput")
    out_d = nc.dram_tensor("out", [S_DEC, D], F32, kind="ExternalOutput")

    nb = 1 if n_iters == 1 else 2

    with tile.TileContext(nc) as tc:
        with (
            tc.tile_pool(name="const", bufs=1) as constp,
            tc.tile_pool(name="inbuf", bufs=nb) as inp,
            tc.tile_pool(name="proj", bufs=nb) as projp,
            tc.tile_pool(name="args", bufs=2) as argsp,
            tc.tile_pool(name="tanh", bufs=2) as tanhp,
            tc.tile_pool(name="post", bufs=nb) as postp,
            tc.tile_pool(name="ps_work", bufs=3, space="PSUM") as ps_work,
            tc.tile_pool(name="ps_sc", bufs=1, space="PSUM") as ps_scp,
        ):
            # ---- constants (outside the timing loop: tiny) --------------
            ident = constp.tile([128, 128], F32)
            make_identity(nc, ident[:])
            wsc_f32 = constp.tile([128, UC], F32)        # [u%128, uc]
            nc.sync.dma_start(
                wsc_f32[:], wsc_d.rearrange("(c p) one -> p (c one)", p=128))
            wsc_bf = constp.tile([128, UC], BF16)
            nc.vector.tensor_copy(wsc_bf[:], wsc_f32[:])
            benc_sb = constp.tile([128, UC], F32)
            nc.sync.dma_start(
                benc_sb[:], benc_d.rearrange("(c p) one -> p (c one)", p=128))
            bdec_sb = constp.tile([128, UC], F32)
            nc.sync.dma_start(
                bdec_sb[:], bdec_d.rearrange("(c p) one -> p (c one)", p=128))

            for _it in range(n_iters):
                # ---- input DMAs -----------------------------------------
                enc_nat = inp.tile([128, EC * D], F32, tag="enc_nat")
                for ec in range(EC):
                    nc.sync.dma_start(enc_nat[:, ec * D:(ec + 1) * D],
                                      enc_d[ec * 128:(ec + 1) * 128, :])
                dec_nat = inp.tile([128, D], F32, tag="dec_nat")
                nc.sync.dma_start(dec_nat[:], dec_d[:])
                wenc_sb = inp.tile([128, DC * U], F32, tag="wenc_sb")
                wdec_sb = inp.tile([128, DC * U], F32, tag="wdec_sb")
                for dc in range(DC):
                    nc.sync.dma_start(wenc_sb[:, dc * U:(dc + 1) * U],
                                      wenc_d[dc * 128:(dc + 1) * 128, :])
                    nc.sync.dma_start(wdec_sb[:, dc * U:(dc + 1) * U],
                                      wdec_d[dc * 128:(dc + 1) * 128, :])

                # ---- transposes: enc_T[d,(dc x e)], dec_T[d,(dc x q)] ---
                enc_t = inp.tile([128, DC * S_ENC], F32, tag="enc_t")
                for dc in range(DC):
                    for ec in range(EC):
                        pst = ps_work.tile([128, 128], F32, tag="ps_work",
                                           name="pst")
                        nc.tensor.transpose(
                            pst[:],
                            enc_nat[:, ec * D + dc * 128: ec * D + dc * 128 + 128],
                            ident[:])
                        nc.vector.tensor_copy(
                            enc_t[:, dc * S_ENC + ec * 128:
                                  dc * S_ENC + ec * 128 + 128],
                            pst[:])
                dec_t = inp.tile([128, DC * 128], F32, tag="dec_t")
                for dc in range(DC):
                    pst = ps_work.tile([128, 128], F32, tag="ps_work",
                                       name="pst")
                    nc.tensor.transpose(
                        pst[:], dec_nat[:, dc * 128:(dc + 1) * 128], ident[:])
                    nc.vector.tensor_copy(
                        dec_t[:, dc * 128:(dc + 1) * 128], pst[:])

                # ---- projections -> transposed --------------------------
                # denc_bf[u%128, (uc, e)] = (enc @ W_enc + b_enc)^T in bf16
                denc_bf = projp.tile([128, UC * S_ENC], BF16, tag="denc_bf")
                for uc in range(UC):
                    psp = ps_work.tile([128, S_ENC], F32, tag="ps_work",
                                       name="psp")
                    for dc in range(DC):
                        nc.tensor.matmul(
                            psp[:],
                            lhsT=wenc_sb[:, dc * U + uc * 128:
                                         dc * U + uc * 128 + 128],
                            rhs=enc_t[:, dc * S_ENC:(dc + 1) * S_ENC],
                            start=(dc == 0), stop=(dc == DC - 1))
                    nc.scalar.activation(
                        denc_bf[:, uc * S_ENC:(uc + 1) * S_ENC], psp[:],
                        AF.Identity, bias=benc_sb[:, uc:uc + 1])
                # ddec_f32[u%128, (uc, q)]; f32: read as tensor_scalar scalars
                ddec_f32 = projp.tile([128, UC * S_DEC], F32, tag="ddec_f32")
                for uc in range(UC):
                    psq = ps_work.tile([128, S_DEC], F32, tag="ps_work",
                                       name="psq")
                    for dc in range(DC):
                        nc.tensor.matmul(
                            psq[:],
                            lhsT=wdec_sb[:, dc * U + uc * 128:
                                         dc * U + uc * 128 + 128],
                            rhs=dec_t[:, dc * 128:(dc + 1) * 128],
                            start=(dc == 0), stop=(dc == DC - 1))
                    nc.scalar.activation(
                        ddec_f32[:, uc * S_DEC:(uc + 1) * S_DEC], psq[:],
                        AF.Identity, bias=bdec_sb[:, uc:uc + 1])

                # ---- main loop: tanh 4D block + score reduction ---------
                # scores^T PSUM tiles [e%128, q], accumulated over uc
                sct = [ps_scp.tile([128, S_DEC], F32, tag=f"sct{ec}",
                                   name=f"sct{ec}")
                       for ec in range(EC)]
                for blk in range(NBLK):
                    args = argsp.tile([128, QB * UC * S_ENC], BF16, tag="args")
                    for ql in range(QB):
                        q = blk * QB + ql
                        for uc in range(UC):
                            nc.vector.tensor_scalar_add(
                                args[:, (ql * UC + uc) * S_ENC:
                                     (ql * UC + uc + 1) * S_ENC],
                                denc_bf[:, uc * S_ENC:(uc + 1) * S_ENC],
                                ddec_f32[:, uc * S_DEC + q:
                                         uc * S_DEC + q + 1])
                    th = tanhp.tile([128, QB * UC * S_ENC], BF16, tag="th")
                    nc.scalar.activation(th[:], args[:], AF.Tanh)
                    for ql in range(QB):
                        q = blk * QB + ql
                        for ec in range(EC):
                            for uc in range(UC):
                                nc.tensor.matmul(
                                    sct[ec][:, q:q + 1],
                                    lhsT=th[:, (ql * UC + uc) * S_ENC + ec * 128:
                                            (ql * UC + uc) * S_ENC + ec * 128 + 128],
                                    rhs=wsc_bf[:, uc:uc + 1],
                                    start=(uc == 0), stop=(uc == UC - 1))

                # ---- softmax over e -------------------------------------
                sct_sb = postp.tile([128, S_ENC], F32, tag="sct_sb")
                for ec in range(EC):
                    nc.vector.tensor_copy(
                        sct_sb[:, ec * 128:(ec + 1) * 128], sct[ec][:])
                sc_ps = ps_work.tile([128, S_ENC], F32, tag="ps_work",
                                     name="sc_ps")
                for ec in range(EC):
                    nc.tensor.transpose(
                        sc_ps[:, ec * 128:(ec + 1) * 128],
                        sct_sb[:, ec * 128:(ec + 1) * 128], ident[:])
                neg_max = postp.tile([128, 1], F32, tag="neg_max")
                nc.vector.tensor_reduce(
                    neg_max[:], sc_ps[:], axis=mybir.AxisListType.X,
                    op=mybir.AluOpType.max, negate=True)
                exp_sb = postp.tile([128, S_ENC], F32, tag="exp_sb")
                nc.scalar.activation(exp_sb[:], sc_ps[:], AF.Exp,
                                     bias=neg_max[:, 0:1])
                ssum = postp.tile([128, 1], F32, tag="ssum")
                nc.vector.tensor_reduce(
                    ssum[:], exp_sb[:], axis=mybir.AxisListType.X,
                    op=mybir.AluOpType.add)
                srec = postp.tile([128, 1], F32, tag="srec")
                nc.vector.reciprocal(srec[:], ssum[:])
                wts = postp.tile([128, S_ENC], F32, tag="wts")
                nc.vector.tensor_scalar_mul(wts[:], exp_sb[:], srec[:, 0:1])

                # ---- context = weights @ enc ----------------------------
                wts_t = postp.tile([128, S_ENC], F32, tag="wts_t")
                for ec in range(EC):
                    pst2 = ps_work.tile([128, 128], F32, tag="ps_work",
                                        name="pst2")
                    nc.tensor.transpose(
                        pst2[:], wts[:, ec * 128:(ec + 1) * 128], ident[:])
                    nc.vector.tensor_copy(
                        wts_t[:, ec * 128:(ec + 1) * 128], pst2[:])
                ctx_ps = ps_work.tile([128, D], F32, tag="ps_work",
                                      name="ctx_ps")
                for ec in range(EC):
                    nc.tensor.matmul(
                        ctx_ps[:],
                        lhsT=wts_t[:, ec * 128:(ec + 1) * 128],
                        rhs=enc_nat[:, ec * D:(ec + 1) * D],
                        start=(ec == 0), stop=(ec == EC - 1))
                out_sb = postp.tile([128, D], F32, tag="out_sb")
                nc.scalar.activation(out_sb[:], ctx_ps[:], AF.Copy)
                nc.sync.dma_start(out_d[:], out_sb[:])

    nc.compile()
    return nc


_CACHED = {}


def _get_program(n_iters: int = 1):
    if n_iters not in _CACHED:
        _CACHED[n_iters] = build_program(n_iters)
    return _CACHED[n_iters]


def _make_in_maps(encodings, decodings, W_enc, W_dec, W_score,
                  bias_enc, bias_dec):
    enc = np.ascontiguousarray(np.asarray(encodings, dtype=np.float32))
    dec = np.ascontiguousarray(np.asarray(decodings, dtype=np.float32))
    com = {
        "w_enc": np.ascontiguousarray(np.asarray(W_enc, dtype=np.float32)),
        "w_dec": np.ascontiguousarray(np.asarray(W_dec, dtype=np.float32)),
        "w_score": np.asarray(W_score, dtype=np.float32).reshape(U, 1),
        "b_enc": np.asarray(bias_enc, dtype=np.float32).reshape(U, 1),
        "b_dec": np.asarray(bias_dec, dtype=np.float32).reshape(U, 1),
    }
    return [{"enc": enc[i], "dec": dec[i], **com} for i in range(N_CORES)]


def run(n_iters=1, **inputs):
    nc = _get_program(n_iters)
    in_maps = _make_in_maps(
        inputs["encodings"], inputs["decodings"], inputs["W_enc"],
        inputs["W_dec"], inputs["W_score"], inputs["bias_enc"],
        inputs["bias_dec"])
    res = run_bass_kernel_spmd(nc, in_maps, list(range(N_CORES)))
    return np.stack([res.results[i]["out"] for i in range(N_CORES)], axis=0)


def kernel(encodings, decodings, W_enc, W_dec, W_score,
           bias_enc, bias_dec, bias_score):
    # bias_score shifts all scores equally and cancels in the softmax.
    del bias_score
    return run(1, encodings=encodings, decodings=decodings, W_enc=W_enc,
               W_dec=W_dec, W_score=W_score, bias_enc=bias_enc,
               bias_dec=bias_dec)
